# revision 17
# baseline (speedup 1.0000x reference)
"""BoundaryDiceLoss Trainium2 kernel (v2).

Full inputs: pred (32,5,512,512) f32, target (32,512,512) int. Output: scalar f32 loss.

Strategy: pure data-parallel over batch across 8 NeuronCores (4 images each).
Strip layout: partition p holds image rows [p*S, p*S+S), so vertical neighbor
diffs are free-dim shifts of an overlap-loaded strip (no cross-partition
shifts, no extra shifted HBM loads). Per image:
  - ONE pred DMA + one overlapped target DMA (plus 2 small edge-row DMAs).
  - softmax via ACT exp + ln/exp reciprocal (both in one act table).
  - per-(b,c) sums: DVE tensor_tensor for products, DVE tensor_scalar
    (fast mode, immediate scalars, is_eq masks / fused add-reduce) for
    masks and counts, ACT Copy+accum for the big plane sums. The Pool
    engine is avoided for plane ops (real gpsimd is ~5-15x slower than
    the cost models suggest; measured via repeat-differencing).
Host combines the per-core [6, BL*C] sums into the final scalar.
"""
import sys

sys.path.insert(0, "/opt/trn_rl_repo")

import numpy as np

NUM_CLASSES = 5
BOUNDARY_WEIGHT = 0.8
EPS = 1e-6
N_CORES = 8

_CACHE = {}


def _build(BL, C, H, W, repeat=1):
    """Build + compile the per-core program.

    Accumulator layout: 6 family tiles (S1, S2, S4h, S5h, K, Wr) of
    [128, BL*C] f32, column = b*C + c; output "sums" [6, BL*C] f32.
    S4h/S5h come out half-scale (boundary weight map is stored as 0.5).
    """
    import concourse.bacc as bacc
    import concourse.tile as tile
    import concourse.mybir as mybir
    import bass_rust

    AF = mybir.ActivationFunctionType
    OP = mybir.AluOpType
    f32 = mybir.dt.float32
    bf16 = mybir.dt.bfloat16
    i32 = mybir.dt.int32

    S = H // 128          # rows per partition strip
    FW = S * W            # free size of one image plane
    OVW = (S + 2) * W     # overlapped target strip free size

    nc = bacc.Bacc("TRN2", target_bir_lowering=False, debug=False)
    pred_d = nc.dram_tensor("pred", [BL, C, H, W], f32, kind="ExternalInput").ap()
    targ_d = nc.dram_tensor("target", [BL, H, W], i32, kind="ExternalInput").ap()
    sums_d = nc.dram_tensor("sums", [6, BL * C], f32, kind="ExternalOutput").ap()

    with tile.TileContext(nc) as tc:
        with (
            tc.tile_pool(name="ptv", bufs=2) as ptv,
            tc.tile_pool(name="px", bufs=1) as px,
            tc.tile_pool(name="pb", bufs=1) as pb,
            tc.tile_pool(name="pP", bufs=2) as pP,
            tc.tile_pool(name="pM", bufs=3) as pM,
            tc.tile_pool(name="pQ", bufs=1) as pQ,
            tc.tile_pool(name="pacc", bufs=1) as pacc,
        ):
            acc = [
                pacc.tile([128, BL * C], f32, tag=f"acc{i}", name=f"acc{i}")
                for i in range(6)
            ]
            for a in acc:
                nc.vector.memset(a[:], 0.0)
            A_S1, A_S2, A_S4, A_S5, A_K, A_W = acc

            # hbuf: horizontal diffs, persistent zero guard columns 0 and W
            # (pad to W+2 so both shifted reads stay in-tile).
            hbuf = pacc.tile([128, S, W + 2], bf16, tag="hbuf", name="hbuf")
            nc.vector.memset(hbuf[:, :, 0:1], 0.0)
            nc.vector.memset(hbuf[:, :, W : W + 2], 0.0)

            for b in [bb for _ in range(repeat) for bb in range(BL)]:
                # ---- target strip with 2-row overlap:
                # tv[p, j, :] = image row p*S - 1 + j   (j = 0..S+1)
                tv = ptv.tile([128, S + 2, W], i32, tag="tv")
                tmain = targ_d[b].rearrange("(p s) w -> p s w", p=128)
                nc.sync.dma_start(tv[:, 1 : S + 1], tmain)
                # j=0 row (p*S-1): last row of previous strip, p>=1
                tup = targ_d[b, S - 1 : H - 1].rearrange("(p s) w -> p s w", s=S)
                nc.sync.dma_start(tv[1:128, 0:1], tup[:, 0:1])
                # j=S+1 row (p*S+S): first row of next strip, p<=126
                tdn = targ_d[b, S:H].rearrange("(p s) w -> p s w", s=S)
                nc.sync.dma_start(tv[0:127, S + 1 : S + 2], tdn[:, 0:1])
                # image top/bottom: duplicate the border row so its vertical
                # diff is 0 (erosion pads with 1 => border contributes no
                # boundary).
                nc.sync.dma_start(tv[0:1, 0:1], tmain[0:1, 0:1])
                nc.sync.dma_start(
                    tv[127:128, S + 1 : S + 2], tmain[127:128, S - 1 : S]
                )

                # ---- cast int32 -> bf16 (values 0..4 exact)
                t_bf = pb.tile([128, OVW], bf16, tag="tbf")
                nc.vector.tensor_copy(
                    t_bf[:].rearrange("p (j w) -> p j w", j=S + 2), tv[:]
                )
                M = t_bf[:, W : W + FW]          # mid view: rows p*S..p*S+S-1
                M3 = M.rearrange("p (s w) -> p s w", s=S)

                # ---- pred load (one DMA) + exp
                xc = px.tile([128, C, S, W], f32, tag="xc")
                pv = pred_d[b].rearrange("c (p s) w -> p c s w", p=128)
                # 3 chunks so the first exp can start after ~2/5 of the load
                nc.sync.dma_start(xc[:, 0:2], pv[:, 0:2])
                nc.sync.dma_start(xc[:, 2:4], pv[:, 2:4])
                nc.sync.dma_start(xc[:, 4:5], pv[:, 4:5])
                E = pb.tile([128, C * FW], bf16, tag="E", bufs=2)
                for c in range(C):
                    nc.scalar.activation(
                        E[:, c * FW : (c + 1) * FW],
                        xc[:, c].rearrange("p s w -> p (s w)"),
                        AF.Exp,
                    )

                # ---- softmax denominator s and rn = 1/s
                a01 = pb.tile([128, FW], bf16, tag="a01")
                nc.vector.tensor_tensor(a01[:], E[:, 0:FW], E[:, FW : 2 * FW], op=OP.add)
                a23 = pb.tile([128, FW], bf16, tag="a23")
                nc.vector.tensor_tensor(
                    a23[:], E[:, 2 * FW : 3 * FW], E[:, 3 * FW : 4 * FW], op=OP.add
                )
                nc.vector.tensor_tensor(a01[:], a01[:], a23[:], op=OP.add)
                nc.vector.tensor_tensor(
                    a01[:], a01[:], E[:, 4 * FW : 5 * FW], op=OP.add
                )
                ln_s = pb.tile([128, FW], f32, tag="lns")
                nc.scalar.activation(ln_s[:], a01[:], AF.Ln)
                rn = pb.tile([128, FW], bf16, tag="rn")
                nc.scalar.activation(rn[:], ln_s[:], AF.Exp, scale=-1.0)

                # ---- boundary weight map w' in {0, 0.5}
                # vertical diffs: vcmp[p, j] = (row p*S-1+j != row p*S+j), j=0..S
                vcmp = pb.tile([128, (S + 1) * W], bf16, tag="vcmp")
                nc.vector.tensor_tensor(
                    vcmp[:], t_bf[:, 0 : (S + 1) * W], t_bf[:, W : OVW],
                    op=OP.not_equal,
                )
                # horizontal diffs into guarded hbuf
                nc.vector.tensor_tensor(
                    hbuf[:, :, 1:W], M3[:, :, 0 : W - 1], M3[:, :, 1:W],
                    op=OP.not_equal,
                )
                # anyd = updiff | downdiff | leftdiff | rightdiff
                anyd = pb.tile([128, FW], bf16, tag="anyd")
                nc.vector.tensor_tensor(
                    anyd[:], vcmp[:, 0:FW], vcmp[:, W : W + FW], op=OP.max
                )
                a3 = anyd[:].rearrange("p (s w) -> p s w", s=S)
                nc.vector.tensor_tensor(a3, a3, hbuf[:, :, 0:W], op=OP.max)
                nc.vector.tensor_tensor(a3, a3, hbuf[:, :, 1 : W + 1], op=OP.max)
                # tpos_half = 0.5*(t>0); w' = anyd * tpos_half
                tph = pb.tile([128, FW], bf16, tag="tph")
                nc.vector.tensor_scalar(
                    tph[:], M, 0.0, 0.5, op0=OP.is_gt, op1=OP.mult
                )
                wh = pb.tile([128, FW], bf16, tag="wh")
                nc.vector.tensor_tensor(wh[:], anyd[:], tph[:], op=OP.mult)

                # ---- count families (is_eq + add-reduce, one 4x op each)
                # N[c] = count(t == c)
                for k in range(C - 1):  # N[C-1] = npix - sum(N[:-1]) on host
                    scr = pQ.tile([128, FW], bf16, tag="scr")
                    nc.vector.tensor_scalar(
                        scr[:], M, float(k), None, op0=OP.is_equal, op1=OP.add,
                        accum_out=A_K[:, b * C + k : b * C + k + 1],
                    )
                # Wcnt[c] = count(t == c AND w' == 0.5) via zM = t + w'
                # (reuses the anyd tile; anyd's last reader is the wh mult)
                zM = anyd
                nc.vector.tensor_tensor(zM[:], M, wh[:], op=OP.add)
                for k in range(1, C):
                    scr = pQ.tile([128, FW], bf16, tag="scr")
                    nc.vector.tensor_scalar(
                        scr[:], zM[:], float(k) + 0.5, None, op0=OP.is_equal,
                        op1=OP.add,
                        accum_out=A_W[:, b * C + k : b * C + k + 1],
                    )

                # ---- per-class families
                for c in range(C):
                    cc = b * C + c
                    Ec = E[:, c * FW : (c + 1) * FW]
                    # class mask on Pool (only legal single-op tensor_scalar)
                    mc = pM.tile([128, FW], bf16, tag="mc")
                    nc.vector.tensor_scalar(mc[:], M, float(c), None, op0=OP.is_equal)
                    Pc = pP.tile([128, FW], bf16, tag="Pc")
                    nc.vector.tensor_tensor(Pc[:], Ec, rn[:], op=OP.mult)
                    if c < C - 1:  # S1[C-1] = npix - sum(S1[:-1]) on host
                        scrA = pQ.tile([128, FW], bf16, tag="scrA")
                        nc.scalar.activation(
                            scrA[:], Pc[:], AF.Copy, accum_out=A_S1[:, cc : cc + 1]
                        )
                    qc = pQ.tile([128, FW], bf16, tag="qc")
                    nc.vector.tensor_tensor(qc[:], mc[:], Pc[:], op=OP.mult)
                    scrA = pQ.tile([128, FW], bf16, tag="scrA")
                    nc.scalar.activation(
                        scrA[:], qc[:], AF.Copy, accum_out=A_S2[:, cc : cc + 1]
                    )
                    PWc = pP.tile([128, FW], bf16, tag="PWc")
                    nc.vector.tensor_tensor(PWc[:], Pc[:], wh[:], op=OP.mult)
                    # S4 accumulation rides ACT (Copy exists in every table)
                    scrA = pQ.tile([128, FW], bf16, tag="scrA")
                    nc.scalar.activation(
                        scrA[:], PWc[:], AF.Copy,
                        accum_out=A_S4[:, cc : cc + 1],
                    )
                    if c > 0:  # S5[0] identically 0 (w'=0 where t=0)
                        q5 = pQ.tile([128, FW], bf16, tag="qc")
                        nc.vector.tensor_tensor(q5[:], mc[:], PWc[:], op=OP.mult)
                        scr = pQ.tile([128, FW], bf16, tag="scr")
                        nc.vector.tensor_scalar(
                            scr[:], q5[:], 0.0, None, op0=OP.bypass, op1=OP.add,
                            accum_out=A_S5[:, cc : cc + 1],
                        )

            # ---- cross-partition reduce + store
            for i in range(6):
                red = pacc.tile([128, BL * C], f32, tag=f"red{i}", name=f"red{i}")
                nc.gpsimd.partition_all_reduce(
                    red[:], acc[i][:], channels=128,
                    reduce_op=bass_rust.ReduceOp.add,
                )
                nc.sync.dma_start(sums_d[i : i + 1, :], red[0:1, :])

    nc.compile()
    return nc


def _get_nc(BL, C, H, W, repeat=1):
    key = (BL, C, H, W, repeat)
    if key not in _CACHE:
        _CACHE[key] = _build(BL, C, H, W, repeat)
    return _CACHE[key]


# ---------------------------------------------------------------------------
# host wrapper
# ---------------------------------------------------------------------------


def _finalize(sums_list, BL, C, npix=512 * 512):
    """sums_list: per-core [6, BL*C] arrays -> scalar loss (f64 internally)."""
    A = np.stack([s.reshape(6, BL, C) for s in sums_list]).astype(np.float64)
    A = A.transpose(1, 0, 2, 3).reshape(6, len(sums_list) * BL, C)
    S1, S2, S4h, S5h, N, Wcnt = A
    S1[:, -1] = npix - S1[:, :-1].sum(axis=1)
    N[:, -1] = npix - N[:, :-1].sum(axis=1)
    S4 = 2.0 * S4h          # device stored sum(P*w') with w' = w/2
    S5 = 2.0 * S5h
    M = Wcnt                # count(t==c & w==1) = sum(onehot*w)
    dice_std = (2.0 * S2 + EPS) / (S1 + N + EPS)
    dice_b = (2.0 * S5 + EPS) / (S4 + M + EPS)
    loss_std = 1.0 - dice_std.mean()
    loss_b = 1.0 - dice_b.mean()
    return np.float32(
        (1.0 - BOUNDARY_WEIGHT) * loss_std + BOUNDARY_WEIGHT * loss_b
    )


def kernel(pred, target):
    from concourse.bass_utils import run_bass_kernel_spmd

    pred = np.ascontiguousarray(np.asarray(pred, dtype=np.float32))
    target = np.ascontiguousarray(np.asarray(target).astype(np.int32))
    B, C, H, W = pred.shape
    assert B % N_CORES == 0
    BL = B // N_CORES

    nc = _get_nc(BL, C, H, W)
    in_maps = [
        {
            "pred": pred[i * BL : (i + 1) * BL],
            "target": target[i * BL : (i + 1) * BL],
        }
        for i in range(N_CORES)
    ]
    res = run_bass_kernel_spmd(nc, in_maps, list(range(N_CORES)))
    return _finalize([res.results[i]["sums"] for i in range(N_CORES)], BL, C)


# revision 20
# speedup vs baseline: 1.8521x; 1.8521x over previous
"""BoundaryDiceLoss Trainium2 kernel (v2).

Full inputs: pred (32,5,512,512) f32, target (32,512,512) int. Output: scalar f32 loss.

Strategy: pure data-parallel over batch across 8 NeuronCores (4 images each).
Strip layout: partition p holds image rows [p*S, p*S+S), so vertical neighbor
diffs are free-dim shifts of an overlap-loaded strip (no cross-partition
shifts, no extra shifted HBM loads). Per image:
  - ONE pred DMA + one overlapped target DMA (plus 2 small edge-row DMAs).
  - softmax via ACT exp + ln/exp reciprocal (both in one act table).
  - per-(b,c) sums: DVE tensor_tensor for products, DVE tensor_scalar
    (fast mode, immediate scalars, is_eq masks / fused add-reduce) for
    masks and counts, ACT Copy+accum for the big plane sums. The Pool
    engine is avoided for plane ops (real gpsimd is ~5-15x slower than
    the cost models suggest; measured via repeat-differencing).
Host combines the per-core [6, BL*C] sums into the final scalar.
"""
import sys

sys.path.insert(0, "/opt/trn_rl_repo")

import numpy as np

NUM_CLASSES = 5
BOUNDARY_WEIGHT = 0.8
EPS = 1e-6
N_CORES = 8

_CACHE = {}


def _build(BL, C, H, W, repeat=1):
    """Build + compile the per-core program.

    Accumulator layout: 6 family tiles (S1, S2, S4h, S5h, K, Wr) of
    [128, BL*C] f32, column = b*C + c; output "sums" [6, BL*C] f32.
    S4h/S5h come out half-scale (boundary weight map is stored as 0.5).
    """
    import concourse.bacc as bacc
    import concourse.tile as tile
    import concourse.mybir as mybir
    import bass_rust

    AF = mybir.ActivationFunctionType
    OP = mybir.AluOpType
    f32 = mybir.dt.float32
    bf16 = mybir.dt.bfloat16
    i32 = mybir.dt.int32

    S = H // 128          # rows per partition strip
    FW = S * W            # free size of one image plane
    OVW = (S + 2) * W     # overlapped target strip free size

    nc = bacc.Bacc("TRN2", target_bir_lowering=False, debug=False)
    pred_d = nc.dram_tensor("pred", [BL, C, H, W], f32, kind="ExternalInput").ap()
    targ_d = nc.dram_tensor("target", [BL, H, W], i32, kind="ExternalInput").ap()
    sums_d = nc.dram_tensor("sums", [6, BL * C], f32, kind="ExternalOutput").ap()

    with tile.TileContext(nc) as tc:
        with (
            tc.tile_pool(name="ptv", bufs=2) as ptv,
            tc.tile_pool(name="px", bufs=1) as px,
            tc.tile_pool(name="pb", bufs=1) as pb,
            tc.tile_pool(name="pP", bufs=2) as pP,
            tc.tile_pool(name="pM", bufs=3) as pM,
            tc.tile_pool(name="pQ", bufs=1) as pQ,
            tc.tile_pool(name="pacc", bufs=1) as pacc,
        ):
            acc = [
                pacc.tile([128, BL * C], f32, tag=f"acc{i}", name=f"acc{i}")
                for i in range(6)
            ]
            for a in acc:
                nc.vector.memset(a[:], 0.0)
            A_S1, A_S2, A_S4, A_S5, A_K, A_W = acc

            # hbuf: horizontal diffs, persistent zero guard columns 0 and W
            # (pad to W+2 so both shifted reads stay in-tile).
            hbuf = pacc.tile([128, S, W + 2], bf16, tag="hbuf", name="hbuf")
            nc.vector.memset(hbuf[:, :, 0:1], 0.0)
            nc.vector.memset(hbuf[:, :, W : W + 2], 0.0)

            for b in [bb for _ in range(repeat) for bb in range(BL)]:
                # ---- pred load first (SP serializes DMA issue; exp0 gates
                # the ACT pipeline, so its chunk goes out before the target)
                xc = px.tile([128, C, S, W], f32, tag="xc")
                pv = pred_d[b].rearrange("c (p s) w -> p c s w", p=128)
                nc.sync.dma_start(xc[:, 0:1], pv[:, 0:1])
                nc.sync.dma_start(xc[:, 1:3], pv[:, 1:3])
                nc.sync.dma_start(xc[:, 3:5], pv[:, 3:5])

                # ---- target strip with 2-row overlap:
                # tv[p, j, :] = image row p*S - 1 + j   (j = 0..S+1)
                tv = ptv.tile([128, S + 2, W], i32, tag="tv")
                tmain = targ_d[b].rearrange("(p s) w -> p s w", p=128)
                nc.sync.dma_start(tv[:, 1 : S + 1], tmain)
                # j=0 row (p*S-1): last row of previous strip, p>=1
                tup = targ_d[b, S - 1 : H - 1].rearrange("(p s) w -> p s w", s=S)
                nc.sync.dma_start(tv[1:128, 0:1], tup[:, 0:1])
                # j=S+1 row (p*S+S): first row of next strip, p<=126
                tdn = targ_d[b, S:H].rearrange("(p s) w -> p s w", s=S)
                nc.sync.dma_start(tv[0:127, S + 1 : S + 2], tdn[:, 0:1])
                # image top/bottom: duplicate the border row so its vertical
                # diff is 0 (erosion pads with 1 => border contributes no
                # boundary).
                nc.sync.dma_start(tv[0:1, 0:1], tmain[0:1, 0:1])
                nc.sync.dma_start(
                    tv[127:128, S + 1 : S + 2], tmain[127:128, S - 1 : S]
                )

                # ---- cast int32 -> bf16 (values 0..4 exact)
                t_bf = pb.tile([128, OVW], bf16, tag="tbf")
                nc.vector.tensor_copy(
                    t_bf[:].rearrange("p (j w) -> p j w", j=S + 2), tv[:]
                )
                M = t_bf[:, W : W + FW]          # mid view: rows p*S..p*S+S-1
                M3 = M.rearrange("p (s w) -> p s w", s=S)
                E = pb.tile([128, C * FW], bf16, tag="E", bufs=2)
                for c in range(C):
                    nc.scalar.activation(
                        E[:, c * FW : (c + 1) * FW],
                        xc[:, c].rearrange("p s w -> p (s w)"),
                        AF.Exp,
                    )

                # ---- softmax denominator s and rn = 1/s
                a01 = pb.tile([128, FW], bf16, tag="a01")
                nc.vector.tensor_tensor(a01[:], E[:, 0:FW], E[:, FW : 2 * FW], op=OP.add)
                a23 = pb.tile([128, FW], bf16, tag="a23")
                nc.vector.tensor_tensor(
                    a23[:], E[:, 2 * FW : 3 * FW], E[:, 3 * FW : 4 * FW], op=OP.add
                )
                nc.vector.tensor_tensor(a01[:], a01[:], a23[:], op=OP.add)
                nc.vector.tensor_tensor(
                    a01[:], a01[:], E[:, 4 * FW : 5 * FW], op=OP.add
                )
                ln_s = pb.tile([128, FW], f32, tag="lns")
                nc.scalar.activation(ln_s[:], a01[:], AF.Ln)
                rn = pb.tile([128, FW], bf16, tag="rn")
                nc.scalar.activation(rn[:], ln_s[:], AF.Exp, scale=-1.0)

                # ---- boundary weight map w' in {0, 0.5}
                # vertical diffs: vcmp[p, j] = (row p*S-1+j != row p*S+j), j=0..S
                vcmp = pb.tile([128, (S + 1) * W], bf16, tag="vcmp")
                nc.vector.tensor_tensor(
                    vcmp[:], t_bf[:, 0 : (S + 1) * W], t_bf[:, W : OVW],
                    op=OP.not_equal,
                )
                # horizontal diffs into guarded hbuf
                nc.vector.tensor_tensor(
                    hbuf[:, :, 1:W], M3[:, :, 0 : W - 1], M3[:, :, 1:W],
                    op=OP.not_equal,
                )
                # anyd = updiff | downdiff | leftdiff | rightdiff
                anyd = pb.tile([128, FW], bf16, tag="anyd")
                nc.vector.tensor_tensor(
                    anyd[:], vcmp[:, 0:FW], vcmp[:, W : W + FW], op=OP.max
                )
                a3 = anyd[:].rearrange("p (s w) -> p s w", s=S)
                nc.vector.tensor_tensor(a3, a3, hbuf[:, :, 0:W], op=OP.max)
                nc.vector.tensor_tensor(a3, a3, hbuf[:, :, 1 : W + 1], op=OP.max)
                # tpos_half = 0.5*(t>0); w' = anyd * tpos_half
                tph = pb.tile([128, FW], bf16, tag="tph")
                nc.vector.tensor_scalar(
                    tph[:], M, 0.0, 0.5, op0=OP.is_gt, op1=OP.mult
                )
                wh = pb.tile([128, FW], bf16, tag="wh")
                nc.vector.tensor_tensor(wh[:], anyd[:], tph[:], op=OP.mult)

                # ---- count families (is_eq + add-reduce, one 4x op each)
                # N[c] = count(t == c)
                for k in range(C - 1):  # N[C-1] = npix - sum(N[:-1]) on host
                    scr = pQ.tile([128, FW], bf16, tag="scr")
                    nc.vector.tensor_scalar(
                        scr[:], M, float(k), None, op0=OP.is_equal, op1=OP.add,
                        accum_out=A_K[:, b * C + k : b * C + k + 1],
                    )
                # Wcnt[c] = count(t == c AND w' == 0.5) via zM = t + w'
                # (reuses the anyd tile; anyd's last reader is the wh mult)
                zM = anyd
                nc.vector.tensor_tensor(zM[:], M, wh[:], op=OP.add)
                for k in range(1, C):
                    scr = pQ.tile([128, FW], bf16, tag="scr")
                    nc.vector.tensor_scalar(
                        scr[:], zM[:], float(k) + 0.5, None, op0=OP.is_equal,
                        op1=OP.add,
                        accum_out=A_W[:, b * C + k : b * C + k + 1],
                    )

                # ---- per-class families
                for c in range(C):
                    cc = b * C + c
                    Ec = E[:, c * FW : (c + 1) * FW]
                    # class mask on Pool (only legal single-op tensor_scalar)
                    mc = pM.tile([128, FW], bf16, tag="mc")
                    nc.vector.tensor_scalar(mc[:], M, float(c), None, op0=OP.is_equal)
                    Pc = pP.tile([128, FW], bf16, tag="Pc")
                    nc.vector.tensor_tensor(Pc[:], Ec, rn[:], op=OP.mult)
                    if c < C - 1:  # S1[C-1] = npix - sum(S1[:-1]) on host
                        scrA = pQ.tile([128, FW], bf16, tag="scrA")
                        nc.scalar.activation(
                            scrA[:], Pc[:], AF.Copy, accum_out=A_S1[:, cc : cc + 1]
                        )
                    qc = pQ.tile([128, FW], bf16, tag="qc")
                    nc.vector.tensor_tensor(qc[:], mc[:], Pc[:], op=OP.mult)
                    scrA = pQ.tile([128, FW], bf16, tag="scrA")
                    nc.scalar.activation(
                        scrA[:], qc[:], AF.Copy, accum_out=A_S2[:, cc : cc + 1]
                    )
                    PWc = pP.tile([128, FW], bf16, tag="PWc")
                    nc.vector.tensor_tensor(PWc[:], Pc[:], wh[:], op=OP.mult)
                    # S4 accumulation rides ACT (Copy exists in every table)
                    scrA = pQ.tile([128, FW], bf16, tag="scrA")
                    nc.scalar.activation(
                        scrA[:], PWc[:], AF.Copy,
                        accum_out=A_S4[:, cc : cc + 1],
                    )
                    if c > 0:  # S5[0] identically 0 (w'=0 where t=0)
                        q5 = pQ.tile([128, FW], bf16, tag="qc")
                        nc.vector.tensor_tensor(q5[:], mc[:], PWc[:], op=OP.mult)
                        scr = pQ.tile([128, FW], bf16, tag="scr")
                        nc.vector.tensor_scalar(
                            scr[:], q5[:], 0.0, None, op0=OP.bypass, op1=OP.add,
                            accum_out=A_S5[:, cc : cc + 1],
                        )

            # ---- cross-partition reduce + store
            for i in range(6):
                red = pacc.tile([128, BL * C], f32, tag=f"red{i}", name=f"red{i}")
                nc.gpsimd.partition_all_reduce(
                    red[:], acc[i][:], channels=128,
                    reduce_op=bass_rust.ReduceOp.add,
                )
                nc.sync.dma_start(sums_d[i : i + 1, :], red[0:1, :])

    nc.compile()
    return nc


def _get_nc(BL, C, H, W, repeat=1):
    key = (BL, C, H, W, repeat)
    if key not in _CACHE:
        _CACHE[key] = _build(BL, C, H, W, repeat)
    return _CACHE[key]


# ---------------------------------------------------------------------------
# host wrapper
# ---------------------------------------------------------------------------


def _finalize(sums_list, BL, C, npix=512 * 512):
    """sums_list: per-core [6, BL*C] arrays -> scalar loss (f64 internally)."""
    A = np.stack([s.reshape(6, BL, C) for s in sums_list]).astype(np.float64)
    A = A.transpose(1, 0, 2, 3).reshape(6, len(sums_list) * BL, C)
    S1, S2, S4h, S5h, N, Wcnt = A
    S1[:, -1] = npix - S1[:, :-1].sum(axis=1)
    N[:, -1] = npix - N[:, :-1].sum(axis=1)
    S4 = 2.0 * S4h          # device stored sum(P*w') with w' = w/2
    S5 = 2.0 * S5h
    M = Wcnt                # count(t==c & w==1) = sum(onehot*w)
    dice_std = (2.0 * S2 + EPS) / (S1 + N + EPS)
    dice_b = (2.0 * S5 + EPS) / (S4 + M + EPS)
    loss_std = 1.0 - dice_std.mean()
    loss_b = 1.0 - dice_b.mean()
    return np.float32(
        (1.0 - BOUNDARY_WEIGHT) * loss_std + BOUNDARY_WEIGHT * loss_b
    )


def kernel(pred, target):
    from concourse.bass_utils import run_bass_kernel_spmd

    pred = np.ascontiguousarray(np.asarray(pred, dtype=np.float32))
    target = np.ascontiguousarray(np.asarray(target).astype(np.int32))
    B, C, H, W = pred.shape
    assert B % N_CORES == 0
    BL = B // N_CORES

    nc = _get_nc(BL, C, H, W)
    in_maps = [
        {
            "pred": pred[i * BL : (i + 1) * BL],
            "target": target[i * BL : (i + 1) * BL],
        }
        for i in range(N_CORES)
    ]
    res = run_bass_kernel_spmd(nc, in_maps, list(range(N_CORES)))
    return _finalize([res.results[i]["sums"] for i in range(N_CORES)], BL, C)
